# revision 1
# baseline (speedup 1.0000x reference)
"""Head-parallel HGNN attention-coefficient kernel for Trainium2 (Bass/Tile).

Per head h (8 heads):
    Q = emb_dest @ Wq[h] + bq[h]            [4096, 512]
    K = emb_src  @ Wk[h] + bk[h]            [4096, 512]
    V = feat_src @ Wv[h] + bv[h]            [4096, 512]
    S = Q @ K^T / sqrt(512)                 [4096, 4096]
    O = elu(softmax(S, -1) @ V)             [4096, 512]
output = mean_h O                           [4096, 512]

Sharding: one head per NeuronCore (8 heads, 8 cores, zero redundant
compute, no collectives). The host transposes emb/feat once (shared by
all cores), casts matmul operands to bf16, and slices per-head weights;
the device computes Q^T/K^T (hidden dim on partitions) so the score
matmul produces S^T tiles (N_src on partitions) whose exp() feeds the
P@V matmul directly as the stationary operand — no on-device transpose
of the 4096x4096 attention matrix.  Softmax runs without max
subtraction (|S|max ~ 2.4 for this problem's distribution, exp is safe)
and the row normalizer is recovered with a ones-vector partition-sum
matmul; normalization and ELU are applied to the [4096, 512] output
tiles.  The host averages the 8 per-head outputs.
"""

import numpy as np
import ml_dtypes

P = 128
D = 512            # IN_DIM
E = 512            # HIDDEN
N = 4096           # N_DST
M = 4096           # N_SRC
H = 8
DC = D // P        # 4 contraction chunks for projections
EC = E // P        # 4
MC = M // P        # 32 N_src chunks
NSTRIP = 512       # N_dst columns handled per strip
NSTRIPS = N // NSTRIP
NCH = NSTRIP // P  # 4 N_dst chunks per strip
SCALE = 1.0 / float(np.sqrt(E))

_cache = {}


def _build_nc(repeat=1):
    import concourse.mybir as mybir
    import concourse.tile as tile
    from concourse import bacc

    f32 = mybir.dt.float32
    f16 = mybir.dt.float16
    AF = mybir.ActivationFunctionType
    ALU = mybir.AluOpType

    nc = bacc.Bacc(
        "TRN2",
        target_bir_lowering=False,
        debug=False,
        enable_asserts=False,
        num_devices=H,
    )

    embT_d_h = nc.dram_tensor("embT_dest", [D, N], f16, kind="ExternalInput")
    embT_s_h = nc.dram_tensor("embT_src", [D, M], f16, kind="ExternalInput")
    featT_h = nc.dram_tensor("featT_src", [E, M], f16, kind="ExternalInput")
    wq_h = nc.dram_tensor("Wq", [D, E], f16, kind="ExternalInput")
    wk_h = nc.dram_tensor("Wk", [D, E], f16, kind="ExternalInput")
    wv_h = nc.dram_tensor("Wv", [E, E], f16, kind="ExternalInput")
    bq_h = nc.dram_tensor("bq", [E], f32, kind="ExternalInput")
    bk_h = nc.dram_tensor("bk", [E], f32, kind="ExternalInput")
    bv_h = nc.dram_tensor("bv", [E], f16, kind="ExternalInput")
    out_h = nc.dram_tensor("out", [N, E], f32, kind="ExternalOutput")

    embT_d = embT_d_h.ap().rearrange("(c p) n -> p c n", p=P)
    embT_s = embT_s_h.ap().rearrange("(c p) n -> p c n", p=P)
    featT = featT_h.ap().rearrange("(c p) n -> p c n", p=P)
    out_ap = out_h.ap()

    with tile.TileContext(nc) as tc:
        with (
            tc.tile_pool(name="wpool", bufs=1) as wpool,
            tc.tile_pool(name="cpool", bufs=1) as cpool,
            tc.tile_pool(name="big", bufs=1) as big_pool,
            tc.tile_pool(name="embx", bufs=6) as embx_pool,
            tc.tile_pool(name="pt", bufs=40) as pt_pool,
            tc.tile_pool(name="ep", bufs=4) as ep_pool,
            tc.tile_pool(name="sm", bufs=2) as sm_pool,
            tc.tile_pool(name="psA", bufs=4, space="PSUM") as psA,
            tc.tile_pool(name="psO", bufs=2, space="PSUM") as psO,
            tc.tile_pool(name="psSm", bufs=1, space="PSUM") as psSm,
            tc.tile_pool(name="psRt", bufs=1, space="PSUM") as psRt,
        ):
            # --- constants / weights ---
            wq_sb = wpool.tile([P, DC, E], f16, name="wq_sb")
            nc.sync.dma_start(wq_sb[:], wq_h.ap().rearrange("(c p) e -> p c e", p=P))
            wk_sb = wpool.tile([P, DC, E], f16, name="wk_sb")
            nc.sync.dma_start(wk_sb[:], wk_h.ap().rearrange("(c p) e -> p c e", p=P))
            wv_sb = wpool.tile([P, EC, E], f16, name="wv_sb")
            nc.sync.dma_start(wv_sb[:], wv_h.ap().rearrange("(c p) e -> p c e", p=P))
            bq_sb = cpool.tile([P, EC], f32, name="bq_sb")
            nc.sync.dma_start(bq_sb[:], bq_h.ap().rearrange("(c p) -> p c", p=P))
            bk_sb = cpool.tile([P, EC], f32, name="bk_sb")
            nc.sync.dma_start(bk_sb[:], bk_h.ap().rearrange("(c p) -> p c", p=P))
            bv_sb = cpool.tile([1, E], f16, name="bv_sb")
            nc.sync.dma_start(bv_sb[:], bv_h.ap().rearrange("(o e) -> o e", o=1))

            ones_row = cpool.tile([1, P], f16, name="ones_row")
            nc.any.memset(ones_row[:], 1.0)
            ones_col = cpool.tile([P, 1], f32, name="ones_col")
            nc.any.memset(ones_col[:], 1.0)
            one_one = cpool.tile([1, 1], f32, name="one_one")
            nc.any.memset(one_one[:], 1.0)

            # (repeat > 1 re-runs the whole computation; used only by the
            # test harness to measure per-iteration HW time differentially)
            for _rep in range(repeat):
                # --- persistent activations ---
                qt_sb = big_pool.tile([P, EC, N], f16, tag="qt", name="qt_sb")
                kt_sb = big_pool.tile([P, EC, M], f16, tag="kt", name="kt_sb")
                v_sb = big_pool.tile([P, MC, E], f16, tag="v", name="v_sb")

                # --- projections: Q^T = Wq^T @ embT_dest, K^T likewise ---
                for src_ap, w_sb, b_sb, dst in (
                    (embT_d, wq_sb, bq_sb, qt_sb),
                    (embT_s, wk_sb, bk_sb, kt_sb),
                ):
                    for nt in range(N // NSTRIP):
                        et = embx_pool.tile(
                            [P, DC, NSTRIP], f16, tag="embx", name="et"
                        )
                        nc.sync.dma_start(
                            et[:], src_ap[:, :, nt * NSTRIP : (nt + 1) * NSTRIP]
                        )
                        for ec in range(EC):
                            ps = psA.tile([P, NSTRIP], f32, tag="psA", name="ps")
                            for dc in range(DC):
                                nc.tensor.matmul(
                                    ps[:],
                                    lhsT=w_sb[:, dc, ec * P : (ec + 1) * P],
                                    rhs=et[:, dc, :],
                                    start=(dc == 0),
                                    stop=(dc == DC - 1),
                                )
                            nc.scalar.activation(
                                dst[:, ec, nt * NSTRIP : (nt + 1) * NSTRIP],
                                ps[:],
                                AF.Identity,
                                bias=b_sb[:, ec : ec + 1],
                            )

                # --- projection: V = feat_src @ Wv + bv (bias as K=1 matmul) ---
                for mt in range(M // NSTRIP):
                    ft = embx_pool.tile([P, EC, NSTRIP], f16, tag="embx", name="ft")
                    nc.sync.dma_start(
                        ft[:], featT[:, :, mt * NSTRIP : (mt + 1) * NSTRIP]
                    )
                    for mi in range(NSTRIP // P):
                        mc = mt * (NSTRIP // P) + mi
                        ps = psA.tile([P, E], f32, tag="psA", name="psv")
                        for ec in range(EC):
                            nc.tensor.matmul(
                                ps[:],
                                lhsT=ft[:, ec, mi * P : (mi + 1) * P],
                                rhs=wv_sb[:, ec, :],
                                start=(ec == 0),
                                stop=False,
                            )
                        nc.tensor.matmul(
                            ps[:],
                            lhsT=ones_row[:],
                            rhs=bv_sb[:],
                            start=False,
                            stop=True,
                        )
                        nc.scalar.activation(v_sb[:, mc, :], ps[:], AF.Copy)

                # --- attention, one strip of 512 N_dst columns at a time ---
                for st in range(NSTRIPS):
                    n0 = st * NSTRIP
                    acc = sm_pool.tile([P, NSTRIP], f32, tag="acc", name="acc")
                    pts = []
                    for mc in range(MC):
                        ps = psA.tile([P, NSTRIP], f32, tag="psA", name="pss")
                        for ec in range(EC):
                            nc.tensor.matmul(
                                ps[:],
                                lhsT=kt_sb[:, ec, mc * P : (mc + 1) * P],
                                rhs=qt_sb[:, ec, n0 : n0 + NSTRIP],
                                start=(ec == 0),
                                stop=(ec == EC - 1),
                            )
                        ptt = pt_pool.tile([P, NSTRIP], f16, tag="pt", name="ptt")
                        nc.scalar.activation(ptt[:], ps[:], AF.Exp, scale=SCALE)
                        pts.append(ptt)
                        # running partition-parallel sum of exp (softmax denom)
                        if mc == 0:
                            nc.vector.tensor_copy(acc[:], ptt[:])
                        else:
                            nc.vector.tensor_add(acc[:], acc[:], ptt[:])

                    # denominators: column-sum over partitions, move n onto
                    # partitions via K=1 matmuls, then reciprocal
                    cs_ps = psSm.tile([1, NSTRIP], f32, tag="cs", name="cs_ps")
                    nc.tensor.matmul(
                        cs_ps[:], lhsT=ones_col[:], rhs=acc[:], start=True, stop=True
                    )
                    cs_sb = sm_pool.tile([1, NSTRIP], f32, tag="cs_sb", name="cs_sb")
                    nc.vector.tensor_copy(cs_sb[:], cs_ps[:])
                    rt_ps = psRt.tile([P, NCH], f32, tag="rt", name="rt_ps")
                    for ncn in range(NCH):
                        nc.tensor.matmul(
                            rt_ps[:, ncn : ncn + 1],
                            lhsT=cs_sb[:, ncn * P : (ncn + 1) * P],
                            rhs=one_one[:],
                            start=True,
                            stop=True,
                        )
                    rinv = sm_pool.tile([P, NCH], f32, tag="rinv", name="rinv")
                    nc.vector.reciprocal(rinv[:], rt_ps[:])

                    # O tile = sum_m exp(S^T)[m, n-chunk]^T @ V[m, :]
                    for ncn in range(NCH):
                        po = psO.tile([P, E], f32, tag="psO", name="po")
                        for mc in range(MC):
                            nc.tensor.matmul(
                                po[:],
                                lhsT=pts[mc][:, ncn * P : (ncn + 1) * P],
                                rhs=v_sb[:, mc, :],
                                start=(mc == 0),
                                stop=(mc == MC - 1),
                            )
                        # normalize + ELU: elu(x) = max(x,0) + min(exp(x),1) - 1
                        t0 = ep_pool.tile([P, E], f32, tag="t0", name="t0")
                        nc.vector.tensor_scalar_mul(
                            t0[:], po[:], rinv[:, ncn : ncn + 1]
                        )
                        ex = ep_pool.tile([P, E], f32, tag="ex", name="ex")
                        nc.scalar.activation(ex[:], t0[:], AF.Exp)
                        nc.vector.tensor_scalar_max(t0[:], t0[:], 0.0)
                        nc.vector.tensor_scalar(
                            ex[:], ex[:], 1.0, -1.0, ALU.min, ALU.add
                        )
                        nc.vector.tensor_add(t0[:], t0[:], ex[:])
                        nc.sync.dma_start(
                            out_ap[n0 + ncn * P : n0 + (ncn + 1) * P, :], t0[:]
                        )

    nc.compile()
    return nc


def _get_nc():
    nc = _cache.get("nc")
    if nc is None:
        nc = _build_nc()
        _cache["nc"] = nc
    return nc


def _make_in_maps(inputs):
    bf = np.float16
    f32 = np.float32
    embT_d = np.asarray(inputs["emb_dest"], f32).T.astype(bf)
    embT_s = np.asarray(inputs["emb_src"], f32).T.astype(bf)
    featT = np.asarray(inputs["feat_src"], f32).T.astype(bf)
    Wq = np.asarray(inputs["Wq"], f32)
    Wk = np.asarray(inputs["Wk"], f32)
    Wv = np.asarray(inputs["Wv"], f32)
    bq = np.asarray(inputs["bq"], f32)
    bk = np.asarray(inputs["bk"], f32)
    bv = np.asarray(inputs["bv"], f32)
    in_maps = []
    for h in range(H):
        in_maps.append(
            {
                "embT_dest": embT_d,
                "embT_src": embT_s,
                "featT_src": featT,
                "Wq": Wq[h].astype(bf),
                "Wk": Wk[h].astype(bf),
                "Wv": Wv[h].astype(bf),
                "bq": np.ascontiguousarray(bq[h]),
                "bk": np.ascontiguousarray(bk[h]),
                "bv": bv[h].astype(bf),
            }
        )
    return in_maps


def kernel(**inputs):
    from concourse.bass_utils import run_bass_kernel_spmd

    nc = _get_nc()
    in_maps = _make_in_maps(inputs)
    res = run_bass_kernel_spmd(nc, in_maps, core_ids=list(range(H)))
    outs = np.stack([r["out"] for r in res.results], axis=0)
    return outs.mean(axis=0, dtype=np.float64).astype(np.float32)



# revision 6
# speedup vs baseline: 4.8242x; 4.8242x over previous
"""Head-parallel HGNN attention-coefficient kernel for Trainium2 (Bass/Tile).

Per head h (8 heads):
    Q = emb_dest @ Wq[h] + bq[h]            [4096, 512]
    K = emb_src  @ Wk[h] + bk[h]            [4096, 512]
    V = feat_src @ Wv[h] + bv[h]            [4096, 512]
    S = Q @ K^T / sqrt(512)                 [4096, 4096]
    O = elu(softmax(S, -1) @ V)             [4096, 512]
output = mean_h O                           [4096, 512]

Sharding: one head per NeuronCore (8 heads, 8 cores, zero redundant
compute, no collectives).  The device computes Q^T/K^T (hidden dim on
partitions) so the score matmul produces S^T tiles (N_src on
partitions) whose exp() feeds the P@V matmul directly as the stationary
operand — no on-device transpose of the 4096x4096 attention matrix.

The two O(N^2 * E) matmuls (scores and P@V) run in fp8e4 with
perf_mode=DoubleRow (256-deep contraction per instruction, ~1.44x bf16
throughput); projections stay f16.  Q^T/K^T/V and exp(S^T) are written
directly as fp8 by the ScalarE activations.  Softmax runs without max
subtraction (|S|max ~ 2.4, exp is in [0.09, 12] — comfortably inside
fp8e4 range) and the row normalizer is recovered with a ones-vector
partition-sum matmul; normalization is folded into the ELU activations
as a per-partition scale.  The host averages the 8 per-head outputs.
"""

import numpy as np

P = 128
D = 512            # IN_DIM
E = 512            # HIDDEN
N = 4096           # N_DST
M = 4096           # N_SRC
H = 8
DC = D // P        # 4 contraction chunks for projections
EC = E // P        # 4
MC = M // P        # 32 N_src chunks
MCP = MC // 2      # 16 N_src chunk pairs (DoubleRow)
NSTRIP = 512       # N_dst columns handled per strip
NSTRIPS = N // NSTRIP
NCH = NSTRIP // P  # 4 N_dst chunks per strip
SCALE = 1.0 / float(np.sqrt(E))

_cache = {}


def _build_nc(repeat=1):
    import concourse.mybir as mybir
    import concourse.tile as tile
    from concourse import bacc

    f32 = mybir.dt.float32
    f16 = mybir.dt.float16
    f8 = mybir.dt.float8e4
    AF = mybir.ActivationFunctionType
    ALU = mybir.AluOpType
    DR = mybir.MatmulPerfMode.DoubleRow

    nc = bacc.Bacc(
        "TRN2",
        target_bir_lowering=False,
        debug=False,
        enable_asserts=False,
        num_devices=H,
    )

    embT_d_h = nc.dram_tensor("embT_dest", [D, N], f16, kind="ExternalInput")
    embT_s_h = nc.dram_tensor("embT_src", [D, M], f16, kind="ExternalInput")
    featT_h = nc.dram_tensor("featT_src", [E, M], f16, kind="ExternalInput")
    wq_h = nc.dram_tensor("Wq", [D, E], f16, kind="ExternalInput")
    wk_h = nc.dram_tensor("Wk", [D, E], f16, kind="ExternalInput")
    wv_h = nc.dram_tensor("Wv", [E, E], f16, kind="ExternalInput")
    bq_h = nc.dram_tensor("bq", [E], f32, kind="ExternalInput")
    bk_h = nc.dram_tensor("bk", [E], f32, kind="ExternalInput")
    bv_h = nc.dram_tensor("bv", [E], f16, kind="ExternalInput")
    out_h = nc.dram_tensor("out", [N, E], f32, kind="ExternalOutput")

    embT_d = embT_d_h.ap().rearrange("(c p) n -> p c n", p=P)
    embT_s = embT_s_h.ap().rearrange("(c p) n -> p c n", p=P)
    featT = featT_h.ap().rearrange("(c p) n -> p c n", p=P)
    out_ap = out_h.ap()

    with tile.TileContext(nc) as tc:
        with (
            tc.tile_pool(name="wpool", bufs=1) as wpool,
            tc.tile_pool(name="cpool", bufs=1) as cpool,
            tc.tile_pool(name="big", bufs=1) as big_pool,
            tc.tile_pool(name="embx", bufs=6) as embx_pool,
            tc.tile_pool(name="pt", bufs=32) as pt_pool,
            tc.tile_pool(name="ep", bufs=4) as ep_pool,
            tc.tile_pool(name="sm", bufs=2) as sm_pool,
            tc.tile_pool(name="psA", bufs=2, space="PSUM") as psA,
            tc.tile_pool(name="psO", bufs=2, space="PSUM") as psO,
            tc.tile_pool(name="psSm", bufs=1, space="PSUM") as psSm,
            tc.tile_pool(name="psRt", bufs=1, space="PSUM") as psRt,
        ):
            # --- constants / weights ---
            wq_sb = wpool.tile([P, DC, E], f16, name="wq_sb")
            nc.sync.dma_start(wq_sb[:], wq_h.ap().rearrange("(c p) e -> p c e", p=P))
            wk_sb = wpool.tile([P, DC, E], f16, name="wk_sb")
            nc.sync.dma_start(wk_sb[:], wk_h.ap().rearrange("(c p) e -> p c e", p=P))
            wv_sb = wpool.tile([P, EC, E], f16, name="wv_sb")
            nc.sync.dma_start(wv_sb[:], wv_h.ap().rearrange("(c p) e -> p c e", p=P))
            bq_sb = cpool.tile([P, EC], f32, name="bq_sb")
            nc.sync.dma_start(bq_sb[:], bq_h.ap().rearrange("(c p) -> p c", p=P))
            bk_sb = cpool.tile([P, EC], f32, name="bk_sb")
            nc.sync.dma_start(bk_sb[:], bk_h.ap().rearrange("(c p) -> p c", p=P))
            bv_sb = cpool.tile([1, E], f16, name="bv_sb")
            nc.sync.dma_start(bv_sb[:], bv_h.ap().rearrange("(o e) -> o e", o=1))

            ones_row = cpool.tile([1, P], f16, name="ones_row")
            nc.any.memset(ones_row[:], 1.0)
            ones_col = cpool.tile([P, 1], f16, name="ones_col")
            nc.any.memset(ones_col[:], 1.0)
            one_one = cpool.tile([1, 1], f32, name="one_one")
            nc.any.memset(one_one[:], 1.0)

            # (repeat > 1 re-runs the whole computation; used only by the
            # test harness to measure per-iteration HW time differentially)
            for _rep in range(repeat):
                # --- persistent activations (fp8 for DoubleRow matmuls) ---
                qt_sb = big_pool.tile([P, EC, N], f8, tag="qt", name="qt_sb")
                kt_sb = big_pool.tile([P, EC, M], f8, tag="kt", name="kt_sb")
                v_sb = big_pool.tile([P, MC, E], f8, tag="v", name="v_sb")

                # --- projections: Q^T = Wq^T @ embT_dest, K^T likewise ---
                for src_ap, w_sb, b_sb, dst in (
                    (embT_d, wq_sb, bq_sb, qt_sb),
                    (embT_s, wk_sb, bk_sb, kt_sb),
                ):
                    for nt in range(N // NSTRIP):
                        et = embx_pool.tile(
                            [P, DC, NSTRIP], f16, tag="embx", name="et"
                        )
                        nc.sync.dma_start(
                            et[:], src_ap[:, :, nt * NSTRIP : (nt + 1) * NSTRIP]
                        )
                        for ecp in range(EC // 2):
                            ps = psA.tile([P, 2, NSTRIP], f32, tag="psA", name="ps")
                            for j in range(2):
                                ec = 2 * ecp + j
                                for dc in range(DC):
                                    nc.tensor.matmul(
                                        ps[:, j, :],
                                        lhsT=w_sb[:, dc, ec * P : (ec + 1) * P],
                                        rhs=et[:, dc, :],
                                        start=(dc == 0),
                                        stop=(dc == DC - 1),
                                    )
                                nc.scalar.activation(
                                    dst[:, ec, nt * NSTRIP : (nt + 1) * NSTRIP],
                                    ps[:, j, :],
                                    AF.Identity,
                                    bias=b_sb[:, ec : ec + 1],
                                )

                # --- projection: V = feat_src @ Wv + bv (bias as K=1 matmul) ---
                for mt in range(M // NSTRIP):
                    ft = embx_pool.tile([P, EC, NSTRIP], f16, tag="embx", name="ft")
                    nc.sync.dma_start(
                        ft[:], featT[:, :, mt * NSTRIP : (mt + 1) * NSTRIP]
                    )
                    for mip in range(NSTRIP // P // 2):
                        ps = psA.tile([P, 2, E], f32, tag="psA", name="psv")
                        for j in range(2):
                            mi = 2 * mip + j
                            mc = mt * (NSTRIP // P) + mi
                            for ec in range(EC):
                                nc.tensor.matmul(
                                    ps[:, j, :],
                                    lhsT=ft[:, ec, mi * P : (mi + 1) * P],
                                    rhs=wv_sb[:, ec, :],
                                    start=(ec == 0),
                                    stop=False,
                                )
                            nc.tensor.matmul(
                                ps[:, j, :],
                                lhsT=ones_row[:],
                                rhs=bv_sb[:],
                                start=False,
                                stop=True,
                            )
                            nc.scalar.activation(v_sb[:, mc, :], ps[:, j, :], AF.Copy)

                # --- attention, one strip of 512 N_dst columns at a time ---
                for st in range(NSTRIPS):
                    n0 = st * NSTRIP
                    # acc2 holds two running partition-parallel exp sums
                    acc2 = sm_pool.tile([P, 2, NSTRIP], f32, tag="acc", name="acc2")
                    pts = []
                    for mcp in range(MCP):
                        ps = psA.tile([P, 2, NSTRIP], f32, tag="psA", name="pss")
                        for j in range(2):
                            mc = 2 * mcp + j
                            for ecp in range(2):
                                nc.tensor.matmul(
                                    ps[:, j, :],
                                    lhsT=kt_sb[
                                        :, 2 * ecp : 2 * ecp + 2, mc * P : (mc + 1) * P
                                    ],
                                    rhs=qt_sb[:, 2 * ecp : 2 * ecp + 2, n0 : n0 + NSTRIP],
                                    start=(ecp == 0),
                                    stop=(ecp == 1),
                                    perf_mode=DR,
                                )
                        ptt = pt_pool.tile([P, 2, NSTRIP], f8, tag="pt", name="ptt")
                        nc.scalar.activation(ptt[:], ps[:], AF.Exp, scale=SCALE)
                        pts.append(ptt)
                        # running partition-parallel sum of exp (softmax denom)
                        if mcp == 0:
                            nc.vector.tensor_copy(acc2[:], ptt[:])
                        else:
                            nc.vector.tensor_add(acc2[:], acc2[:], ptt[:])

                    # denominators: fold acc2 pair, column-sum over partitions
                    # (f16 ones matmul), move n onto partitions via K=1
                    # matmuls, then reciprocal
                    acc_bf = sm_pool.tile([P, NSTRIP], f16, tag="accbf", name="acc_bf")
                    nc.vector.tensor_add(acc_bf[:], acc2[:, 0, :], acc2[:, 1, :])
                    cs_ps = psSm.tile([1, NSTRIP], f32, tag="cs", name="cs_ps")
                    nc.tensor.matmul(
                        cs_ps[:], lhsT=ones_col[:], rhs=acc_bf[:], start=True, stop=True
                    )
                    cs_sb = sm_pool.tile([1, NSTRIP], f32, tag="cs_sb", name="cs_sb")
                    nc.vector.tensor_copy(cs_sb[:], cs_ps[:])
                    rt_ps = psRt.tile([P, NCH], f32, tag="rt", name="rt_ps")
                    for ncn in range(NCH):
                        nc.tensor.matmul(
                            rt_ps[:, ncn : ncn + 1],
                            lhsT=cs_sb[:, ncn * P : (ncn + 1) * P],
                            rhs=one_one[:],
                            start=True,
                            stop=True,
                        )
                    rinv = sm_pool.tile([P, NCH], f32, tag="rinv", name="rinv")
                    nc.vector.reciprocal(rinv[:], rt_ps[:])

                    # O tile = sum_m exp(S^T)[m, n-chunk]^T @ V[m, :]
                    for ncn in range(NCH):
                        po = psO.tile([P, E], f32, tag="psO", name="po")
                        for mcp in range(MCP):
                            nc.tensor.matmul(
                                po[:],
                                lhsT=pts[mcp][:, :, ncn * P : (ncn + 1) * P],
                                rhs=v_sb[:, 2 * mcp : 2 * mcp + 2, :],
                                start=(mcp == 0),
                                stop=(mcp == MCP - 1),
                                perf_mode=DR,
                            )
                        # normalize + ELU: elu(x) = max(x,0) + min(exp(x),1) - 1
                        # with x = po * rinv (per-partition scale on ScalarE)
                        rv = rinv[:, ncn : ncn + 1]
                        ex = ep_pool.tile([P, E], f32, tag="ex", name="ex")
                        nc.scalar.activation(ex[:], po[:], AF.Exp, scale=rv)
                        t0 = ep_pool.tile([P, E], f32, tag="t0", name="t0")
                        nc.scalar.activation(t0[:], po[:], AF.Relu, scale=rv)
                        nc.vector.tensor_scalar(
                            ex[:], ex[:], 1.0, -1.0, ALU.min, ALU.add
                        )
                        nc.vector.tensor_add(t0[:], t0[:], ex[:])
                        nc.sync.dma_start(
                            out_ap[n0 + ncn * P : n0 + (ncn + 1) * P, :], t0[:]
                        )

    nc.compile()
    return nc


def _get_nc():
    nc = _cache.get("nc")
    if nc is None:
        nc = _build_nc()
        _cache["nc"] = nc
    return nc


def _make_in_maps(inputs):
    bf = np.float16
    f32 = np.float32
    embT_d = np.asarray(inputs["emb_dest"], f32).T.astype(bf)
    embT_s = np.asarray(inputs["emb_src"], f32).T.astype(bf)
    featT = np.asarray(inputs["feat_src"], f32).T.astype(bf)
    Wq = np.asarray(inputs["Wq"], f32)
    Wk = np.asarray(inputs["Wk"], f32)
    Wv = np.asarray(inputs["Wv"], f32)
    bq = np.asarray(inputs["bq"], f32)
    bk = np.asarray(inputs["bk"], f32)
    bv = np.asarray(inputs["bv"], f32)
    in_maps = []
    for h in range(H):
        in_maps.append(
            {
                "embT_dest": embT_d,
                "embT_src": embT_s,
                "featT_src": featT,
                "Wq": Wq[h].astype(bf),
                "Wk": Wk[h].astype(bf),
                "Wv": Wv[h].astype(bf),
                "bq": np.ascontiguousarray(bq[h]),
                "bk": np.ascontiguousarray(bk[h]),
                "bv": bv[h].astype(bf),
            }
        )
    return in_maps


def kernel(**inputs):
    from concourse.bass_utils import run_bass_kernel_spmd

    nc = _get_nc()
    in_maps = _make_in_maps(inputs)
    res = run_bass_kernel_spmd(nc, in_maps, core_ids=list(range(H)))
    outs = np.stack([r["out"] for r in res.results], axis=0)
    return outs.mean(axis=0, dtype=np.float64).astype(np.float32)


# revision 10
# speedup vs baseline: 5.3708x; 1.1133x over previous
"""Head-parallel HGNN attention-coefficient kernel for Trainium2 (Bass/Tile).

Per head h (8 heads):
    Q = emb_dest @ Wq[h] + bq[h]            [4096, 512]
    K = emb_src  @ Wk[h] + bk[h]            [4096, 512]
    V = feat_src @ Wv[h] + bv[h]            [4096, 512]
    S = Q @ K^T / sqrt(512)                 [4096, 4096]
    O = elu(softmax(S, -1) @ V)             [4096, 512]
output = mean_h O                           [4096, 512]

One head per NeuronCore; no collectives; host averages the 8 outputs.

All O(N^2) matmuls run in fp8e4 DoubleRowSwInterleave (256-deep
contraction per instruction, ~129ns vs 169ns for plain DoubleRow: the
pre-interleaved weight layout keeps the fast weight-load path).  The
stationary operands are produced directly in the interleaved-reversed
layout: Wq/Wk are interleaved on the host; K^T and exp(S^T) are written
interleaved on-device via negative-stride APs.  The softmax denominator
is a ones-vector DR matmul over a stride-2 view of the interleaved
exp tiles (sums the same fp8 values P@V consumes).  V projection stays
f16 for accuracy.

The instruction stream is software-pipelined: the scores/exp production
of strip s+1 is interleaved slot-by-slot with the P@V consumption of
strip s, so the in-order PE and ScalarE queues overlap instead of
alternating.  Q-projection runs two strips ahead inside the steady
loop; K and V projections form the prologue.
"""

import numpy as np

P = 128
D = 512            # IN_DIM
E = 512            # HIDDEN
N = 4096           # N_DST
M = 4096           # N_SRC
H = 8
DC = D // P        # 4 contraction chunks for projections
EC = E // P        # 4
MC = M // P        # 32 N_src chunks
MCP = MC // 2      # 16 N_src chunk pairs (DoubleRow)
NSTRIP = 512       # N_dst columns handled per strip
NSTRIPS = N // NSTRIP
NCH = NSTRIP // P  # 4 N_dst chunks per strip
WSCALE = 16.0      # host pre-scale on Wq/Wk/bq/bk (fp8 subnormal dodge)
SCALE = 1.0 / (float(np.sqrt(E)) * WSCALE * WSCALE)

_cache = {}


def _build_nc(repeat=1):
    import concourse.mybir as mybir
    import concourse.tile as tile
    from concourse import bacc

    f32 = mybir.dt.float32
    f16 = mybir.dt.float16
    f8 = mybir.dt.float8e4
    AF = mybir.ActivationFunctionType
    ALU = mybir.AluOpType
    DRI = mybir.MatmulPerfMode.DoubleRowSwInterleave
    DR = mybir.MatmulPerfMode.DoubleRow

    nc = bacc.Bacc(
        "TRN2",
        target_bir_lowering=False,
        debug=False,
        enable_asserts=False,
        num_devices=H,
    )

    embT_d_h = nc.dram_tensor("embT_dest", [D, N], f8, kind="ExternalInput")
    embT_s_h = nc.dram_tensor("embT_src", [D, M], f8, kind="ExternalInput")
    featT_h = nc.dram_tensor("featT_src", [E, M], f16, kind="ExternalInput")
    # host-interleaved DRI weights: [p, dcp, ec, 2*(127-u)+i]
    wq_h = nc.dram_tensor("Wqi", [P, DC // 2, EC * 2 * P], f8, kind="ExternalInput")
    wk_h = nc.dram_tensor("Wki", [P, DC // 2, EC * 2 * P], f8, kind="ExternalInput")
    wv_h = nc.dram_tensor("Wv", [E, E], f16, kind="ExternalInput")
    bq_h = nc.dram_tensor("bq", [E], f32, kind="ExternalInput")
    bk_h = nc.dram_tensor("bk", [E], f32, kind="ExternalInput")
    bv_h = nc.dram_tensor("bv", [E], f16, kind="ExternalInput")
    out_h = nc.dram_tensor("out", [N, E], f32, kind="ExternalOutput")

    embT_d = embT_d_h.ap().rearrange("(c p) n -> p c n", p=P)
    embT_s = embT_s_h.ap().rearrange("(c p) n -> p c n", p=P)
    featT = featT_h.ap().rearrange("(c p) n -> p c n", p=P)
    out_ap = out_h.ap()

    with tile.TileContext(nc) as tc:
        with (
            tc.tile_pool(name="wpool", bufs=1) as wpool,
            tc.tile_pool(name="cpool", bufs=1) as cpool,
            tc.tile_pool(name="big", bufs=1) as big_pool,
            tc.tile_pool(name="embx", bufs=4) as embx_pool,
            tc.tile_pool(name="pt", bufs=32) as pt_pool,
            tc.tile_pool(name="ep", bufs=4) as ep_pool,
            tc.tile_pool(name="sm", bufs=2) as sm_pool,
            tc.tile_pool(name="psA", bufs=4, space="PSUM") as psA,
            tc.tile_pool(name="psO", bufs=2, space="PSUM") as psO,
            tc.tile_pool(name="psSm", bufs=1, space="PSUM") as psSm,
            tc.tile_pool(name="psRt", bufs=1, space="PSUM") as psRt,
        ):
            # --- constants / weights ---
            wq_sb = wpool.tile([P, DC // 2, EC, 2 * P], f8, name="wq_sb")
            nc.sync.dma_start(
                wq_sb[:], wq_h.ap().rearrange("p c (e u) -> p c e u", e=EC)
            )
            wk_sb = wpool.tile([P, DC // 2, EC, 2 * P], f8, name="wk_sb")
            nc.sync.dma_start(
                wk_sb[:], wk_h.ap().rearrange("p c (e u) -> p c e u", e=EC)
            )
            wv_sb = wpool.tile([P, EC, E], f16, name="wv_sb")
            nc.sync.dma_start(wv_sb[:], wv_h.ap().rearrange("(c p) e -> p c e", p=P))
            bq_sb = cpool.tile([P, EC], f32, name="bq_sb")
            nc.sync.dma_start(bq_sb[:], bq_h.ap().rearrange("(c p) -> p c", p=P))
            bk_sb = cpool.tile([P, EC], f32, name="bk_sb")
            nc.sync.dma_start(bk_sb[:], bk_h.ap().rearrange("(c p) -> p c", p=P))
            bv_sb = cpool.tile([1, E], f16, name="bv_sb")
            nc.sync.dma_start(bv_sb[:], bv_h.ap().rearrange("(o e) -> o e", o=1))

            ones_row = cpool.tile([1, P], f16, name="ones_row")
            nc.any.memset(ones_row[:], 1.0)
            # fp8 ones pair for the DR denominator matmul (pair stride 16B)
            ones8 = cpool.tile([P, 2, 16], f8, name="ones8")
            nc.any.memset(ones8[:], 1.0)
            one_one = cpool.tile([1, 1], f32, name="one_one")
            nc.any.memset(one_one[:], 1.0)

            for _rep in range(repeat):
                # --- persistent activations ---
                qt_sb = big_pool.tile([P, EC, N], f8, tag="qt", name="qt_sb")
                # interleaved K^T, one tile per ec-pair: [p, mc, 2*(127-u)+i]
                kt_i = [
                    big_pool.tile([P, MC, 2 * P], f8, tag=f"kt{ecp}", name=f"kt{ecp}")
                    for ecp in range(EC // 2)
                ]
                v_sb = big_pool.tile([P, MC, E], f8, tag="v", name="v_sb")

                def q_proj(nt):
                    """Q^T strip nt: DRI matmuls + DVE bias-add (fp8 store)."""
                    et = embx_pool.tile([P, DC, NSTRIP], f8, tag="embq", name="etq")
                    nc.sync.dma_start(
                        et[:], embT_d[:, :, nt * NSTRIP : (nt + 1) * NSTRIP]
                    )
                    for ec in range(EC):
                        ps = psA.tile([P, NSTRIP], f32, tag="psA", name="psq")
                        for dcp in range(DC // 2):
                            nc.tensor.matmul(
                                ps[:],
                                lhsT=wq_sb[:, dcp, ec, :],
                                rhs=et[:, 2 * dcp : 2 * dcp + 2, :],
                                start=(dcp == 0),
                                stop=(dcp == DC // 2 - 1),
                                perf_mode=DRI,
                            )
                        nc.vector.tensor_scalar_add(
                            qt_sb[:, ec, nt * NSTRIP : (nt + 1) * NSTRIP],
                            ps[:],
                            bq_sb[:, ec : ec + 1],
                        )

                # ---------- prologue: K^T (interleaved) and V ----------
                for nt in range(M // NSTRIP):
                    et = embx_pool.tile([P, DC, NSTRIP], f8, tag="embq", name="etk")
                    nc.sync.dma_start(
                        et[:], embT_s[:, :, nt * NSTRIP : (nt + 1) * NSTRIP]
                    )
                    for ecp in range(EC // 2):
                        for j in range(2):
                            ec = 2 * ecp + j
                            ps = psA.tile([P, NCH, P], f32, tag="psA", name="psk")
                            for dcp in range(DC // 2):
                                nc.tensor.matmul(
                                    ps[:],
                                    lhsT=wk_sb[:, dcp, ec, :],
                                    rhs=et[:, 2 * dcp : 2 * dcp + 2, :],
                                    start=(dcp == 0),
                                    stop=(dcp == DC // 2 - 1),
                                    perf_mode=DRI,
                                )
                            # interleaved-reversed store: addr = 2*(127-u)+j
                            nc.scalar.activation(
                                kt_i[ecp][
                                    :, nt * NCH : (nt + 1) * NCH, (2 * P - 2 + j) :: -2
                                ],
                                ps[:],
                                AF.Identity,
                                bias=bk_sb[:, ec : ec + 1],
                            )

                for mt in range(M // NSTRIP):
                    ft = embx_pool.tile([P, EC, NSTRIP], f16, tag="embv", name="ft")
                    nc.sync.dma_start(
                        ft[:], featT[:, :, mt * NSTRIP : (mt + 1) * NSTRIP]
                    )
                    for mi in range(NSTRIP // P):
                        mc = mt * (NSTRIP // P) + mi
                        ps = psA.tile([P, E], f32, tag="psA", name="psv")
                        for ec in range(EC):
                            nc.tensor.matmul(
                                ps[:],
                                lhsT=ft[:, ec, mi * P : (mi + 1) * P],
                                rhs=wv_sb[:, ec, :],
                                start=(ec == 0),
                                stop=False,
                            )
                        nc.tensor.matmul(
                            ps[:],
                            lhsT=ones_row[:],
                            rhs=bv_sb[:],
                            start=False,
                            stop=True,
                        )
                        # split V stores between ScalarE and DVE
                        if mc % 2 == 0:
                            nc.scalar.activation(v_sb[:, mc, :], ps[:], AF.Copy)
                        else:
                            nc.vector.tensor_copy(v_sb[:, mc, :], ps[:])

                q_proj(0)
                q_proj(1)

                # ---------- software-pipelined strip loop ----------
                # state carried between periods
                pts_prev = None      # pt tiles of strip s (consumed by PV)
                rinv_prev = None     # 1/denominator of strip s

                for period in range(NSTRIPS + 1):
                    sp = period          # strip whose scores/exp are produced
                    sc = period - 1      # strip whose PV/ELU are consumed
                    n0 = sp * NSTRIP
                    do_scores = sp < NSTRIPS
                    do_pv = sc >= 0

                    pts_new = []
                    cs_ps = (
                        psSm.tile([1, NCH, P], f32, tag="cs", name="cs_ps")
                        if do_scores
                        else None
                    )
                    pos = [psO.tile([P, E], f32, tag="psO", name=f"po{k}") for k in range(2)] if do_pv else None

                    def emit_denom(mcp):
                        # denominator: DR ones-matmul over the stride-2 plane
                        # view of the interleaved exp tile (same fp8 values
                        # the P@V matmul consumes)
                        nc.tensor.matmul(
                            cs_ps[:],
                            lhsT=ones8[:, :, 0:1],
                            rhs=pts_new[mcp][:]
                            .rearrange("p b u -> p (b u)")
                            .rearrange("p (x two) -> p two x", two=2),
                            start=(mcp == 0),
                            stop=(mcp == MCP - 1),
                            perf_mode=DR,
                        )

                    for k in range(MCP):
                        if do_scores:
                            mcp = k
                            ptt = pt_pool.tile([P, NCH, 2 * P], f8, tag="pt", name="ptt")
                            for j in range(2):
                                mc = 2 * mcp + j
                                ps = psA.tile([P, NCH, P], f32, tag="psA", name="pss")
                                for ecp in range(2):
                                    nc.tensor.matmul(
                                        ps[:],
                                        lhsT=kt_i[ecp][:, mc, :],
                                        rhs=qt_sb[
                                            :, 2 * ecp : 2 * ecp + 2, n0 : n0 + NSTRIP
                                        ],
                                        start=(ecp == 0),
                                        stop=(ecp == 1),
                                        perf_mode=DRI,
                                    )
                                # exp -> interleaved-reversed fp8 store
                                nc.scalar.activation(
                                    ptt[:, :, (2 * P - 2 + j) :: -2],
                                    ps[:],
                                    AF.Exp,
                                    scale=SCALE,
                                )
                            pts_new.append(ptt)
                            # lag the denominator matmul 2 pairs behind the
                            # exps so the in-order PE never waits on ScalarE
                            if k >= 2:
                                emit_denom(k - 2)

                        if do_pv:
                            ncn = k // NCH
                            po = pos[ncn % 2]
                            for mm in range(NCH):
                                mcp = (k % NCH) * NCH + mm
                                nc.tensor.matmul(
                                    po[:],
                                    lhsT=pts_prev[mcp][:, ncn, :],
                                    rhs=v_sb[:, 2 * mcp : 2 * mcp + 2, :],
                                    start=(mcp == 0),
                                    stop=(mcp == MCP - 1),
                                    perf_mode=DRI,
                                )
                            if k % NCH == NCH - 1:
                                # normalize + ELU:
                                # elu(x) = max(x,0) + min(exp(x)-1, 0),
                                # x = po * rinv (per-partition scale)
                                rv = rinv_prev[:, ncn : ncn + 1]
                                ex = ep_pool.tile([P, E], f32, tag="ex", name="ex")
                                nc.scalar.activation(ex[:], po[:], AF.Exp, scale=rv)
                                t0 = ep_pool.tile([P, E], f32, tag="t0", name="t0")
                                nc.vector.tensor_scalar(
                                    t0[:], po[:], rv, 0.0, ALU.mult, ALU.max
                                )
                                nc.vector.tensor_scalar(
                                    ex[:], ex[:], -1.0, 0.0, ALU.add, ALU.min
                                )
                                nc.vector.tensor_add(t0[:], t0[:], ex[:])
                                nc.sync.dma_start(
                                    out_ap[
                                        sc * NSTRIP + ncn * P : sc * NSTRIP + (ncn + 1) * P,
                                        :,
                                    ],
                                    t0[:],
                                )

                        if k == 7 and sp + 2 < NSTRIPS:
                            q_proj(sp + 2)

                    if do_scores:
                        emit_denom(MCP - 2)
                        emit_denom(MCP - 1)
                        # cs_raw2[b, f] holds denom of n = b*128 + (127-f):
                        # un-reverse while copying PSUM -> SBUF
                        cs_sb = sm_pool.tile([1, NCH, P], f32, tag="cs_sb", name="cs_sb")
                        nc.vector.tensor_copy(
                            cs_sb[:, :, (P - 1) :: -1], cs_ps[:]
                        )
                        rt_ps = psRt.tile([P, NCH], f32, tag="rt", name="rt_ps")
                        for ncn in range(NCH):
                            nc.tensor.matmul(
                                rt_ps[:, ncn : ncn + 1],
                                lhsT=cs_sb[0:1, ncn, :],
                                rhs=one_one[:],
                                start=True,
                                stop=True,
                            )
                        rinv = sm_pool.tile([P, NCH], f32, tag="rinv", name="rinv")
                        nc.vector.reciprocal(rinv[:], rt_ps[:])
                        rinv_prev = rinv
                        pts_prev = pts_new

    nc.compile()
    return nc


def _get_nc():
    nc = _cache.get("nc")
    if nc is None:
        nc = _build_nc()
        _cache["nc"] = nc
    return nc


def _interleave_w(w):
    """[D, E] -> DRI layout [p, dcp, ec*256 + 2*(127-u)+i]."""
    import ml_dtypes

    D_, E_ = w.shape
    wr = w.reshape(DC // 2, 2, P, EC, P)          # [dcp, i, p, ec, u]
    wr = wr[:, :, :, :, ::-1]                     # u -> 127-u
    wr = wr.transpose(2, 0, 3, 4, 1)              # [p, dcp, ec, u', i]
    return np.ascontiguousarray(wr.reshape(P, DC // 2, EC * 2 * P)).astype(
        ml_dtypes.float8_e4m3
    )


def _make_in_maps(inputs):
    import ml_dtypes

    f8 = ml_dtypes.float8_e4m3
    bf = np.float16
    f32 = np.float32
    embT_d = np.asarray(inputs["emb_dest"], f32).T.astype(f8)
    embT_s = np.asarray(inputs["emb_src"], f32).T.astype(f8)
    featT = np.asarray(inputs["feat_src"], f32).T.astype(bf)
    Wq = np.asarray(inputs["Wq"], f32) * WSCALE
    Wk = np.asarray(inputs["Wk"], f32) * WSCALE
    Wv = np.asarray(inputs["Wv"], f32)
    bq = np.asarray(inputs["bq"], f32) * WSCALE
    bk = np.asarray(inputs["bk"], f32) * WSCALE
    bv = np.asarray(inputs["bv"], f32)
    in_maps = []
    for h in range(H):
        in_maps.append(
            {
                "embT_dest": embT_d,
                "embT_src": embT_s,
                "featT_src": featT,
                "Wqi": _interleave_w(Wq[h]),
                "Wki": _interleave_w(Wk[h]),
                "Wv": Wv[h].astype(bf),
                "bq": np.ascontiguousarray(bq[h]),
                "bk": np.ascontiguousarray(bk[h]),
                "bv": bv[h].astype(bf),
            }
        )
    return in_maps


def kernel(**inputs):
    from concourse.bass_utils import run_bass_kernel_spmd

    nc = _get_nc()
    in_maps = _make_in_maps(inputs)
    res = run_bass_kernel_spmd(nc, in_maps, core_ids=list(range(H)))
    outs = np.stack([r["out"] for r in res.results], axis=0)
    return outs.mean(axis=0, dtype=np.float64).astype(np.float32)
